# revision 55
# baseline (speedup 1.0000x reference)
"""Trainium2 Bass kernel for a 2-layer LSTM encoder/decoder forecaster.

Model (per batch element):
  teacher-forced over S=168 steps:  enc -> LSTM0 -> LSTM1 (keep last out)
  autoregressive rollout for 23 more steps feeding decoder output back.

Sharding: data-parallel, batch 1024 -> 8 cores x 128. 128 = partition width,
so each core's activations are single-partition-tile matrices. All weights
are replicated and stay resident in SBUF; zero inter-core communication.

Layout: gates are batch-major ([B=128, 4H]) with activations as the
stationary matmul operand (feature-major lhsT) and transposed weights as the
moving operand (N=512 chunks, one PSUM bank each). The encoder is fused into
layer 0 (M0 = Wih0 @ W_enc, bias via ones-row). All matmul operands are
bf16 (weights quantized host-side, h cast in the PSUM->SBUF copy; cell
state c and the gate accumulation stay f32) - bf16 runs the PE at 1 row/
cycle with half-cost LDWEIGHTS and transposes, rel err ~4e-3 vs the f32
reference.

Schedule: the tensor-engine queue is ordered so every dependency wait point
(h transposes, h0->layer1 gates) has a slab of independent matmuls in front
of it. Per teacher-forced step the PE stream is
  [x0 | h0part | T(h1[t-1]) | bias1 | h1part(n0..2 k-major) | T(h0[t])
   | h1part(n3) | h0->1]
which keeps the PE busy back-to-back: no idle gaps, so the continuously-busy
PE holds its fast p-state (an idle gap drops the clock for ~3us). The
PSUM->SBUF h copies are chunked per 128 columns alternating DVE/Scalar so
the first LDWEIGHTS waits only on its own chunk; the cell elementwise is
spread over Scalar (activations), DVE (muls/adds) and Pool (f*c product)
with the tanh(c)*sig(o) tail split in halves to start the transpose early.
Decoder steps pre-issue the next step's h0-part/bias/h1-n3 matmuls so the
PE has runway over the serial decoder->x[t+1] chain.
"""

import sys
import threading

sys.path.insert(0, "/opt/trn_rl_repo")

import numpy as np

PRED_LEN = 24
F, I, H = 64, 128, 512
B, S = 1024, 168
NCORES = 8
BL = B // NCORES          # batch per core = 128
G = 4 * H                 # gate width 2048
KX = F + 1                # x operand rows incl. ones row
F2 = F + 2                # decoder width padded to even

_cache = {}
_cache_lock = threading.Lock()


def _gate_perm():
    # pytorch gate order i,f,g,o -> reorder rows to (i,f,o,g) so the three
    # sigmoid gates are contiguous for a single wide ACT op.
    return np.concatenate([
        np.arange(0, H),            # i
        np.arange(H, 2 * H),        # f
        np.arange(3 * H, 4 * H),    # o
        np.arange(2 * H, 3 * H),    # g
    ])


def _build_program(n_tf=S, n_ar=PRED_LEN - 1):
    import concourse.bacc as bacc
    import concourse.tile as tile
    import concourse.mybir as mybir

    F32 = mybir.dt.float32
    F32R = mybir.dt.float32r
    BF16 = mybir.dt.bfloat16
    AF = mybir.ActivationFunctionType

    nc = bacc.Bacc("TRN2", target_bir_lowering=False, debug=False,
                   num_devices=NCORES)

    xT_d = nc.dram_tensor("xT", [n_tf, KX, BL], BF16, kind="ExternalInput").ap()
    m0_d = nc.dram_tensor("m0t", [KX, G], BF16, kind="ExternalInput").ap()
    whh0_d = nc.dram_tensor("whh0t", [H, G], BF16, kind="ExternalInput").ap()
    wih1_d = nc.dram_tensor("wih1t", [H, G], BF16, kind="ExternalInput").ap()
    whh1_d = nc.dram_tensor("whh1t", [H, G], BF16, kind="ExternalInput").ap()
    b1_d = nc.dram_tensor("b1", [1, G], BF16, kind="ExternalInput").ap()
    wdec_d = nc.dram_tensor("wdect", [H, F2], BF16, kind="ExternalInput").ap()
    bdec_d = nc.dram_tensor("bdec", [F2, 1], F32, kind="ExternalInput").ap()
    ones_d = nc.dram_tensor("ones", [1, BL], BF16, kind="ExternalInput").ap()
    ident_d = nc.dram_tensor("ident", [128, 128], BF16, kind="ExternalInput").ap()
    zeros_d = nc.dram_tensor("zeros", [128, H], BF16, kind="ExternalInput").ap()
    y_d = nc.dram_tensor("y", [n_ar + 1, F, BL], BF16, kind="ExternalOutput").ap()

    from contextlib import ExitStack
    with tile.TileContext(nc) as tc, ExitStack() as ctx:
        wpool = ctx.enter_context(tc.tile_pool(name="w", bufs=1))
        xpool = ctx.enter_context(tc.tile_pool(name="x", bufs=4))
        spool = ctx.enter_context(tc.tile_pool(name="s", bufs=2))
        hpool = ctx.enter_context(tc.tile_pool(name="h", bufs=2))
        pspool = ctx.enter_context(tc.tile_pool(name="ps", bufs=3, space="PSUM"))
        tppool = ctx.enter_context(tc.tile_pool(name="tp", bufs=2, space="PSUM"))

        # ---- resident weights ----
        m0_sb = wpool.tile([KX, G], BF16)
        nc.sync.dma_start(m0_sb[:], m0_d[:])
        # [H, G] weights stored k-chunk-major: [128, 4*G]
        whh0_sb = wpool.tile([128, 4 * G], BF16)
        wih1_sb = wpool.tile([128, 4 * G], BF16)
        whh1_sb = wpool.tile([128, 4 * G], BF16)
        for dst, srcd in ((whh0_sb, whh0_d), (wih1_sb, wih1_d), (whh1_sb, whh1_d)):
            for k in range(4):
                nc.sync.dma_start(dst[:, k * G:(k + 1) * G],
                                  srcd[k * 128:(k + 1) * 128, :])
        b1_sb = wpool.tile([1, G], BF16)
        nc.sync.dma_start(b1_sb[:], b1_d[:])
        wdec_sb = wpool.tile([128, 4 * F2], BF16)
        for k in range(4):
            nc.sync.dma_start(wdec_sb[:, k * F2:(k + 1) * F2],
                              wdec_d[k * 128:(k + 1) * 128, :])
        bdec_sb = wpool.tile([F2, 1], F32)
        nc.sync.dma_start(bdec_sb[:], bdec_d[:])
        ones_sb = wpool.tile([1, BL], BF16)
        nc.sync.dma_start(ones_sb[:], ones_d[:])
        ident_sb = wpool.tile([128, 128], BF16)
        nc.sync.dma_start(ident_sb[:], ident_d[:])

        # ---- state ----
        h0T = hpool.tile([128, H], BF16, tag="h0T")
        nc.sync.dma_start(h0T[:], zeros_d[:])
        h1T = hpool.tile([128, H], BF16, tag="h1T")
        nc.sync.dma_start(h1T[:], zeros_d[:])
        c0 = hpool.tile([BL, H], F32, tag="c0")
        nc.gpsimd.memset(c0[:], 0.0)
        c1 = hpool.tile([BL, H], F32, tag="c1")
        nc.gpsimd.memset(c1[:], 0.0)

        tc.strict_bb_all_engine_barrier()

        def psl(gA, gB, n):
            gt = gA if n < 2 else gB
            return gt[:, (n % 2) * H:(n % 2 + 1) * H]

        def mm_x0(gA, gB, xa, start, stop, ns=(0, 1, 2, 3)):
            for n in ns:
                nc.tensor.matmul(psl(gA, gB, n), xa[:],
                                 m0_sb[:, n * H:(n + 1) * H],
                                 start=start, stop=stop)

        def mm_h(gA, gB, hT, w_sb, ns, start, stop, korder=False,
                 ks=(0, 1, 2, 3)):
            # korder: k varies slowest, giving each freshly-copied hT chunk
            # len(ns) matmuls of cover before its LDWEIGHTS is needed
            loops = [(k, n) for k in ks for n in ns] if korder else \
                    [(k, n) for n in ns for k in ks]
            for k, n in loops:
                nc.tensor.matmul(
                    psl(gA, gB, n), hT[:, k * 128:(k + 1) * 128],
                    w_sb[:, k * G + n * H: k * G + (n + 1) * H],
                    start=(start and k == 0), stop=(stop and k == 3))

        def mm_bias(gA, gB, ns=(0, 1, 2, 3)):
            for n in ns:
                nc.tensor.matmul(psl(gA, gB, n), ones_sb[:],
                                 b1_sb[:, n * H:(n + 1) * H],
                                 start=True, stop=False)

        def transpose_h(h_bm):
            tp = tppool.tile([128, H], BF16, tag="tp")
            for k in range(4):
                nc.tensor.transpose(tp[:, k * 128:(k + 1) * 128],
                                    h_bm[:, k * 128:(k + 1) * 128], ident_sb[:])
            return tp

        def copy_hT_split(tp, tag):
            """h1T copy split: chunks 0-1 on DVE immediately (h1part k0/k1
            need them first); chunks 2-3 deferred onto Scalar AFTER tanh_g/
            sig_o so they don't head-of-line-block the cell chain."""
            hT_new = hpool.tile([128, H], BF16, tag=tag)
            for k in (0, 1):
                sl = slice(k * 128, (k + 1) * 128)
                nc.vector.tensor_copy(hT_new[:, sl], tp[:, sl])

            def finish():
                for k in (2, 3):
                    sl = slice(k * 128, (k + 1) * 128)
                    nc.scalar.activation(hT_new[:, sl], tp[:, sl], AF.Copy)
            return hT_new, finish

        def sig_if_act(gA):
            sig_if = spool.tile([BL, 2 * H], F32, tag="sif")
            nc.scalar.activation(sig_if[:], gA[:], AF.Sigmoid)
            return sig_if

        def copy_hT(tp, tag):
            """PSUM->SBUF copy in 128-col chunks, alternating DVE/Scalar
            (GPSIMD can't read PSUM). Chunked so the matmul LDWEIGHTS of
            k-chunk 0 waits only for chunk 0, not the whole tile."""
            hT_new = hpool.tile([128, H], BF16, tag=tag)
            for k in range(4):
                sl = slice(k * 128, (k + 1) * 128)
                if k % 2 == 0:
                    nc.vector.tensor_copy(hT_new[:, sl], tp[:, sl])
                else:
                    nc.scalar.activation(hT_new[:, sl], tp[:, sl], AF.Copy)
            return hT_new

        def cell_rest(gB, sig_if, c_prev, c_tag, h_tag, after_so=None):
            """g/o acts + DVE/Pool state chain: returns (c_new, h_bm).
            tanh(g) runs before sigmoid(o) (g gates the c-chain); the
            tanh(c)*sig(o) tail is split in halves so the transpose of the
            first half starts ~0.7us earlier."""
            tanh_g = spool.tile([BL, H], F32, tag="tg")
            nc.scalar.activation(tanh_g[:], gB[:, H:2 * H], AF.Tanh)
            sig_o = spool.tile([BL, H], F32, tag="so")
            nc.scalar.activation(sig_o[:], gB[:, 0:H], AF.Sigmoid)
            if after_so is not None:
                after_so()
            fc = spool.tile([BL, H], F32, tag="fc")
            nc.gpsimd.tensor_mul(fc[:], sig_if[:, H:2 * H], c_prev[:])
            # halved ig/c/tanh/h chain, first half prioritized so chunk-0 of
            # the transpose+copy (feeding the next LDWEIGHTS) starts early
            ig = spool.tile([BL, H], F32, tag="ig")
            c_new = hpool.tile([BL, H], F32, tag=c_tag)
            tanh_c = spool.tile([BL, H], F32, tag="tc")
            h_bm = spool.tile([BL, H], BF16, tag=h_tag)
            a, b = slice(0, 256), slice(256, 512)
            nc.vector.tensor_mul(ig[:, a], sig_if[:, a], tanh_g[:, a])
            nc.vector.tensor_add(c_new[:, a], ig[:, a], fc[:, a])
            nc.scalar.activation(tanh_c[:, a], c_new[:, a], AF.Tanh)
            nc.vector.tensor_mul(ig[:, b], sig_if[:, b], tanh_g[:, b])
            nc.vector.tensor_mul(h_bm[:, a], sig_o[:, a], tanh_c[:, a])
            nc.vector.tensor_add(c_new[:, b], ig[:, b], fc[:, b])
            nc.scalar.activation(tanh_c[:, b], c_new[:, b], AF.Tanh)
            nc.vector.tensor_mul(h_bm[:, b], sig_o[:, b], tanh_c[:, b])
            return c_new, h_bm

        n_steps = n_tf + n_ar
        xa_t = {}

        def issue_xa(tt):
            if tt < n_tf:
                xa = xpool.tile([KX, BL], BF16, tag="xa")
                nc.sync.dma_start(xa[:], xT_d[tt])
                xa_t[tt] = xa

        for i in range(3):
            issue_xa(i)

        h1_bm_prev = None     # layer-1 batch-major h awaiting transpose
        pend_g0 = None        # (gA0, gB0) pre-opened by previous dec step
        pend_g1 = None        # (gA1, gB1) pre-opened by previous dec step
        xa_next = None        # AR input tile from previous step's decoder

        for t in range(n_steps):
            issue_xa(t + 3)
            dec = t >= n_tf - 1
            prev_dec = t - 1 >= n_tf - 1

            # ---- gates0[t] ----
            if not prev_dec:
                gA0 = pspool.tile([BL, 2 * H], F32, tag="g")
                gB0 = pspool.tile([BL, 2 * H], F32, tag="g")
                mm_x0(gA0, gB0, xa_t.pop(t), start=True, stop=False)
                mm_h(gA0, gB0, h0T, whh0_sb, (0, 1, 3, 2), start=False, stop=True)
            else:
                # h-part was pre-issued (start=True) at the end of the
                # previous body; close the group with the x-part.
                gA0, gB0 = pend_g0
                mm_x0(gA0, gB0, xa_next, start=False, stop=True, ns=(0, 1, 3, 2))
                mm_h(pend_g1[0], pend_g1[1], h1T, whh1_sb, (0, 1, 2),
                     start=False, stop=False, korder=True)

            # sig_if0 first on the Scalar queue so the hT copy doesn't
            # delay the cell0 chain
            sif0 = sig_if_act(gA0)

            # ---- transpose h1[t-1] (TF flow; dec steps did it eagerly) ----
            h1T_finish = None
            if t > 0 and not prev_dec:
                tpb = transpose_h(h1_bm_prev)
                h1T, h1T_finish = copy_hT_split(tpb, "h1T")

            # ---- gates1[t]: bias + h1-part (dec steps pre-issued both) ----
            if not prev_dec:
                gA1 = pspool.tile([BL, 2 * H], F32, tag="g")
                gB1 = pspool.tile([BL, 2 * H], F32, tag="g")
                mm_bias(gA1, gB1)
                mm_h(gA1, gB1, h1T, whh1_sb, (0, 1, 2), start=False, stop=False,
                     korder=True, ks=(0, 1))
            else:
                gA1, gB1 = pend_g1

            # ---- cell0 elementwise (engines wait on data, not issue order) --
            c0, h0_bm = cell_rest(gB0, sif0, c0, "c0", "h0",
                                  after_so=h1T_finish)
            if not prev_dec:
                mm_h(gA1, gB1, h1T, whh1_sb, (0, 1, 2), start=False,
                     stop=False, korder=True, ks=(2, 3))

            # ---- transpose h0[t] ----
            tpa = transpose_h(h0_bm)
            h0T = copy_hT(tpa, "h0T")

            # ---- gates1[t]: h1-part (n3), then h0-part ----
            if not prev_dec:
                mm_h(gA1, gB1, h1T, whh1_sb, (3,), start=False, stop=False)
            mm_h(gA1, gB1, h0T, wih1_sb, (0, 1, 3, 2), start=False, stop=True)

            # ---- cell1 elementwise ----
            sif1 = sig_if_act(gA1)
            c1, h1_bm = cell_rest(gB1, sif1, c1, "c1", "h1")
            h1_bm_prev = h1_bm

            # ---- decoder + AR tail ----
            if dec:
                if t + 1 < n_steps:
                    # pre-open next step's gates0 with the h-part so the PE
                    # has work while the decoder chain produces x[t+1]
                    gA0n = pspool.tile([BL, 2 * H], F32, tag="g")
                    gB0n = pspool.tile([BL, 2 * H], F32, tag="g")
                    mm_h(gA0n, gB0n, h0T, whh0_sb, (0, 1, 2, 3),
                         start=True, stop=False)
                    pend_g0 = (gA0n, gB0n)
                # transpose h1[t] eagerly (the decoder needs h1T[t] now)
                tpb2 = transpose_h(h1_bm)
                h1T = copy_hT(tpb2, "h1T")
                if t + 1 < n_steps:
                    # h1T[t] already exists, so next step's bias + h1-part
                    # can be pre-issued too (more PE runway over the serial
                    # decoder chain)
                    gA1n = pspool.tile([BL, 2 * H], F32, tag="g")
                    gB1n = pspool.tile([BL, 2 * H], F32, tag="g")
                    mm_bias(gA1n, gB1n, (0, 1, 3))
                    mm_h(gA1n, gB1n, h1T, whh1_sb, (3,),
                         start=False, stop=False)
                    mm_bias(gA1n, gB1n, (2,))
                    pend_g1 = (gA1n, gB1n)

                # decoder computed transposed: doutT = wdec^T @ h1T, so the
                # bias is per-partition and the feedback input needs no
                # transpose. wdec's zero-pad row + bdec's 1.0 pad generate
                # the ones-row of xa directly; y is stored [F, BL].
                dpsT = tppool.tile([F2, BL], F32, tag="tp")
                for k in range(4):
                    nc.tensor.matmul(
                        dpsT[:], wdec_sb[:, k * F2:(k + 1) * F2],
                        h1T[:, k * 128:(k + 1) * 128],
                        start=(k == 0), stop=(k == 3))
                xa_next = xpool.tile([KX, BL], BF16, tag="xa")
                nc.scalar.activation(xa_next[:], dpsT[0:KX, :], AF.Identity,
                                     bias=bdec_sb[0:KX, :])
                nc.sync.dma_start(y_d[t - (n_tf - 1)], xa_next[0:F, :])

    nc.compile()
    return nc


def _get_program(n_tf=S, n_ar=PRED_LEN - 1):
    key = (n_tf, n_ar)
    with _cache_lock:
        if key not in _cache:
            _cache[key] = _build_program(n_tf, n_ar)
        return _cache[key]


def _prep_weights(W_enc, b_enc, Wih0, Whh0, bih0, bhh0,
                  Wih1, Whh1, bih1, bhh1, W_dec, b_dec):
    perm = _gate_perm()
    f32 = np.float32

    M0 = (Wih0 @ W_enc)[perm]                                   # [G, F]
    b0 = (Wih0 @ b_enc + bih0 + bhh0)[perm]                     # [G]
    m0t = np.concatenate([M0.T, b0[None, :]], axis=0)           # [KX, G]

    whh0t = np.ascontiguousarray(Whh0[perm].T)                  # [H, G]
    wih1t = np.ascontiguousarray(Wih1[perm].T)                  # [H, G]
    whh1t = np.ascontiguousarray(Whh1[perm].T)                  # [H, G]
    b1 = (bih1 + bhh1)[perm][None, :]                           # [1, G]

    wdect = np.concatenate([W_dec.T, np.zeros((H, 2), f32)], axis=1)  # [H, F2]
    bdec = np.concatenate([b_dec, np.ones((1,), f32), np.zeros((1,), f32)])
    bdec_b = np.ascontiguousarray(bdec[:, None], f32)           # [F2, 1]

    import ml_dtypes
    bf16 = ml_dtypes.bfloat16

    return {
        "m0t": np.ascontiguousarray(m0t).astype(bf16),
        "whh0t": whh0t.astype(bf16),
        "wih1t": wih1t.astype(bf16),
        "whh1t": whh1t.astype(bf16),
        "b1": np.ascontiguousarray(b1).astype(bf16),
        "wdect": np.ascontiguousarray(wdect).astype(bf16),
        "bdec": bdec_b.astype(f32),
        "ones": np.ones((1, BL), bf16),
        "ident": np.eye(128, dtype=bf16),
        "zeros": np.zeros((128, H), bf16),
    }


def _make_in_maps(x, weights, _n_tf=S):
    in_maps = []
    for c in range(NCORES):
        import ml_dtypes
        bf16 = ml_dtypes.bfloat16
        xs = x[c * BL:(c + 1) * BL, :_n_tf, :]                # [BL, n_tf, F]
        xT = np.ascontiguousarray(xs.transpose(1, 2, 0))      # [n_tf, F, BL]
        xa = np.concatenate(
            [xT, np.ones((_n_tf, 1, BL), np.float32)], axis=1)  # [n_tf, KX, BL]
        in_maps.append({"xT": np.ascontiguousarray(xa).astype(bf16), **weights})
    return in_maps


def kernel(x, W_enc, b_enc, Wih0, Whh0, bih0, bhh0,
           Wih1, Whh1, bih1, bhh1, W_dec, b_dec, _n_tf=S, _n_ar=PRED_LEN - 1):
    from concourse.bass_utils import run_bass_kernel_spmd

    x = np.asarray(x, np.float32)
    weights = _prep_weights(
        np.asarray(W_enc, np.float32), np.asarray(b_enc, np.float32),
        np.asarray(Wih0, np.float32), np.asarray(Whh0, np.float32),
        np.asarray(bih0, np.float32), np.asarray(bhh0, np.float32),
        np.asarray(Wih1, np.float32), np.asarray(Whh1, np.float32),
        np.asarray(bih1, np.float32), np.asarray(bhh1, np.float32),
        np.asarray(W_dec, np.float32), np.asarray(b_dec, np.float32))

    nc = _get_program(_n_tf, _n_ar)
    in_maps = _make_in_maps(x, weights, _n_tf)

    res = run_bass_kernel_spmd(nc, in_maps, core_ids=list(range(NCORES)))

    out = np.empty((B, _n_ar + 1, F), np.float32)
    for c in range(NCORES):
        y = np.asarray(res.results[c]["y"], np.float32)       # [n_ar+1, F, BL]
        out[c * BL:(c + 1) * BL] = y.transpose(2, 0, 1)
    return out
